# revision 14
# baseline (speedup 1.0000x reference)
"""Canny edge detection (nn_Canny_56916906606715) on 8 Trainium2 NeuronCores.

Data-parallel over batch: 16 images -> 2 per core. Per core, per image:
  - horizontal convs (gaussian 5-tap, sobel h-parts) as shifted-AP FMAs on VectorE
  - vertical convs as banded fp32 matmuls on TensorE (edge-exact staged band
    matrices, PSUM accumulate)
  - Square/Sqrt/Abs/Sign on ScalarE
  - NMS: sector masks from (GX, GY) sign/tan tests; neighbor max selected via
    copy_predicated; vertical +-1 shifts via SBUF->SBUF DMA copies
Output is the binary 0/1 float mask, exactly mirroring the reference ordering
(threshold >= 2 AND strict local max along the quantized gradient direction).
"""
import numpy as np
from contextlib import ExitStack

import concourse.bass as bass
import concourse.tile as tile
from concourse import mybir
from concourse.bass_utils import run_bass_kernel_spmd

F32 = mybir.dt.float32
AF = mybir.ActivationFunctionType
OP = mybir.AluOpType

H = W = 512
VALID = 122          # valid output rows per conv tile (128 - 2*3)
NT = 5               # conv tiles per image (122*4 + 24 = 512)
NIMG = 2             # images per core
NCH = 3
NCORES = 8
PI_REF = 3.14159     # reference uses this, not np.pi


_WS_CTR = [0]


def _split_excess_waits(nc, limit=1):
    """walrus drains (TPB_CTRL_NO_STRUCT) accept a single sync wait; Tile's
    kernel-tail drain can carry several. Split excess waits onto injected
    same-engine drains placed immediately before."""
    n_fixed = 0
    for f in nc.m.functions:
        for bb in f.blocks:
            insts = bb.instructions
            i = 0
            while i < len(insts):
                ins = insts[i]
                lim = 1
                si = getattr(ins, 'sync_info', None)
                if si is not None and len(si.on_wait) > lim:
                    waits = list(si.on_wait)
                    ins.sync_info = mybir.SyncInfo(
                        on_wait=waits[:lim], on_update=list(si.on_update))
                    rest = waits[lim:]
                    new = []
                    while rest:
                        chunk, rest = rest[:1], rest[1:]
                        d = mybir.InstDrain(
                            name=f"I-waitsplit-{_WS_CTR[0]}", ins=[], outs=[])
                        _WS_CTR[0] += 1
                        d.engine = ins.engine
                        d.sync_info = mybir.SyncInfo(on_wait=chunk, on_update=[])
                        new.append(d)
                    insts[i:i] = new
                    i += len(new)
                    n_fixed += 1
                i += 1
    return n_fixed


def _padded_band(taps, offs, N=H):
    M = np.zeros((N, N), dtype=np.float64)
    for tp, d in zip(taps, offs):
        idx = np.arange(max(0, -d), min(N, N - d))
        M[idx, idx + d] = tp
    return M


def _make_band_consts(gauss_w):
    """Build per-tile lhsT band blocks [128, 2*NT*128] (A blocks then B blocks)."""
    g = np.asarray(gauss_w, dtype=np.float32)
    g1 = np.float64(g[1])
    Gv = _padded_band(g.astype(np.float64), [-2, -1, 0, 1, 2])
    Sm = _padded_band([1, 2, 1], [-1, 0, 1])
    Dv = _padded_band([1, 0, -1], [-1, 0, 1])
    MA = ((Sm @ Gv) * g1).astype(np.float32)   # gx = MA @ Xd   (Xd scaled by 1/g1)
    MB = ((Dv @ Gv) * g1).astype(np.float32)   # gy = MB @ Xs
    out = np.zeros((128, 2 * NT * 128), dtype=np.float32)
    for bi, Mfull in enumerate((MA, MB)):
        for t in range(NT):
            r0 = VALID * t - 3
            L = np.zeros((128, 128), dtype=np.float32)
            for m in range(128):
                ro = r0 + m
                if 0 <= ro < H:
                    ks = np.arange(max(0, -r0), min(128, H - r0))
                    L[ks, m] = Mfull[ro, r0 + ks]
            out[:, (bi * NT + t) * 128:(bi * NT + t + 1) * 128] = L
    return out


def _build_program(gauss_w):
    g = np.asarray(gauss_w, dtype=np.float32)
    c0 = float(np.float32(g[0] / g[1]))
    c2 = float(np.float32(1.0 / g[1]))
    T22 = float(np.float32(np.tan(PI_REF / 8)))

    nc = bass.Bass()
    x = nc.declare_dram_parameter("x", [NIMG, NCH, H, W], F32, isOutput=False)
    bands = nc.declare_dram_parameter("bands", [128, 2 * NT * 128], F32, isOutput=False)
    y = nc.declare_dram_parameter("y", [NIMG, 1, H, W], F32, isOutput=True)

    with tile.TileContext(nc) as tc, ExitStack() as ctx:
        cpool = ctx.enter_context(tc.tile_pool(name="consts", bufs=1))
        bt = cpool.tile([128, 2 * NT * 128], F32)
        nc.gpsimd.dma_start(bt[:], bands[:])

        xpool = ctx.enter_context(tc.tile_pool(name="xin", bufs=3))
        hbpool = ctx.enter_context(tc.tile_pool(name="hb", bufs=2))
        xhpool = ctx.enter_context(tc.tile_pool(name="xh", bufs=2))
        xdpool = ctx.enter_context(tc.tile_pool(name="xd", bufs=2))
        smpool = ctx.enter_context(tc.tile_pool(name="sums", bufs=2))
        sqpool = ctx.enter_context(tc.tile_pool(name="sq", bufs=2))
        gdpool = ctx.enter_context(tc.tile_pool(name="grad", bufs=NT + 1))
        mkpool = ctx.enter_context(tc.tile_pool(name="mask", bufs=NT))
        nmpool = ctx.enter_context(tc.tile_pool(name="nms", bufs=2))
        psA = ctx.enter_context(tc.tile_pool(name="psA", bufs=2, space="PSUM"))
        psB = ctx.enter_context(tc.tile_pool(name="psB", bufs=2, space="PSUM"))

        for img in range(NIMG):
            grads, mhs, mvs, sds = [], [], [], []
            for t in range(NT):
                r0 = VALID * t - 3
                nv = min(VALID, H - VALID * t)
                lo, hi = max(0, r0), min(H, r0 + 128)
                grad = gdpool.tile([128, 514], F32, tag="grad")
                nc.gpsimd.memset(grad[:, 0:1], 0.0)
                nc.gpsimd.memset(grad[:, 513:514], 0.0)
                xds, xss = [], []
                for c in range(NCH):
                    X = xpool.tile([128, 516], F32, tag="X")
                    if t == 0 or t == NT - 1:
                        nc.gpsimd.memset(X[:], 0.0)
                    else:
                        nc.gpsimd.memset(X[:, 0:2], 0.0)
                        nc.gpsimd.memset(X[:, 514:516], 0.0)
                    nc.gpsimd.dma_start(X[lo - r0:hi - r0, 2:514], x[img, c, lo:hi, :])

                    # horizontal gaussian (scaled by 1/g1), zero guard cols
                    t1 = hbpool.tile([128, 512], F32, tag="t1")
                    t2 = hbpool.tile([128, 512], F32, tag="t2")
                    nc.vector.tensor_tensor(t1[:], X[:, 0:512], X[:, 4:516], OP.add)
                    nc.vector.tensor_tensor(t2[:], X[:, 1:513], X[:, 3:515], OP.add)
                    Xh = xhpool.tile([128, 514], F32, tag="Xh")
                    nc.gpsimd.memset(Xh[:, 0:1], 0.0)
                    nc.gpsimd.memset(Xh[:, 513:514], 0.0)
                    u = hbpool.tile([128, 512], F32, tag="u")
                    nc.vector.scalar_tensor_tensor(u[:], t1[:], c0, t2[:], OP.mult, OP.add)
                    nc.vector.scalar_tensor_tensor(
                        Xh[:, 1:513], X[:, 2:514], c2, u[:], OP.mult, OP.add)

                    # sobel horizontal parts
                    Xd = xdpool.tile([128, 512], F32, tag=f"Xd{c}")
                    nc.vector.tensor_tensor(Xd[:], Xh[:, 0:512], Xh[:, 2:514], OP.subtract)
                    Xs = xdpool.tile([128, 512], F32, tag=f"Xs{c}")
                    nc.vector.scalar_tensor_tensor(
                        Xs[:], Xh[:, 1:513], 2.0, Xh[:, 0:512], OP.mult, OP.add)
                    nc.vector.tensor_tensor(Xs[:], Xs[:], Xh[:, 2:514], OP.add)
                    xds.append(Xd)
                    xss.append(Xs)

                    # vertical convs on TensorE (fp32 banded matmuls)
                    gxp = psA.tile([128, 512], F32, tag="gx")
                    nc.tensor.matmul(gxp[:], bt[:, (0 * NT + t) * 128:(0 * NT + t + 1) * 128],
                                     Xd[:], start=True, stop=True)
                    gyp = psA.tile([128, 512], F32, tag="gy")
                    nc.tensor.matmul(gyp[:], bt[:, (1 * NT + t) * 128:(1 * NT + t + 1) * 128],
                                     Xs[:], start=True, stop=True)

                    sqx = sqpool.tile([128, 512], F32, tag="sqx")
                    nc.scalar.activation(sqx[:], gxp[:], AF.Square)
                    sqy = sqpool.tile([128, 512], F32, tag="sqy")
                    nc.scalar.activation(sqy[:], gyp[:], AF.Square)
                    nc.vector.tensor_tensor(sqx[:], sqx[:], sqy[:], OP.add)
                    if c == 0:
                        nc.scalar.activation(grad[:, 1:513], sqx[:], AF.Sqrt)
                    else:
                        mag = sqpool.tile([128, 512], F32, tag="mag")
                        nc.scalar.activation(mag[:], sqx[:], AF.Sqrt)
                        nc.vector.tensor_tensor(grad[:, 1:513], grad[:, 1:513], mag[:], OP.add)

                # channel sums -> GX, GY
                Xdsum = smpool.tile([128, 512], F32, tag="xdsum")
                nc.vector.tensor_tensor(Xdsum[:], xds[0][:], xds[1][:], OP.add)
                nc.vector.tensor_tensor(Xdsum[:], Xdsum[:], xds[2][:], OP.add)
                Xssum = smpool.tile([128, 512], F32, tag="xssum")
                nc.vector.tensor_tensor(Xssum[:], xss[0][:], xss[1][:], OP.add)
                nc.vector.tensor_tensor(Xssum[:], Xssum[:], xss[2][:], OP.add)
                GXp = psB.tile([128, 512], F32, tag="GX")
                nc.tensor.matmul(GXp[:], bt[:, (0 * NT + t) * 128:(0 * NT + t + 1) * 128],
                                 Xdsum[:], start=True, stop=True)
                GYp = psB.tile([128, 512], F32, tag="GY")
                nc.tensor.matmul(GYp[:], bt[:, (1 * NT + t) * 128:(1 * NT + t + 1) * 128],
                                 Xssum[:], start=True, stop=True)

                # sector masks
                ax = sqpool.tile([128, 512], F32, tag="ax")
                nc.scalar.activation(ax[:], GXp[:], AF.Abs)
                ay = sqpool.tile([128, 512], F32, tag="ay")
                nc.scalar.activation(ay[:], GYp[:], AF.Abs)
                sgx = sqpool.tile([128, 512], F32, tag="sgx")
                nc.scalar.activation(sgx[:], GXp[:], AF.Sign)
                s = sqpool.tile([128, 512], F32, tag="s")
                nc.vector.tensor_tensor(s[:], sgx[:], GYp[:], OP.mult)
                sd = mkpool.tile([128, 512], F32, tag="sd")
                nc.vector.tensor_single_scalar(sd[:], s[:], 0.0, OP.is_gt)
                mh = mkpool.tile([128, 512], F32, tag="mh")
                nc.vector.scalar_tensor_tensor(mh[:], ax[:], T22, ay[:], OP.mult, OP.is_ge)
                mv = mkpool.tile([128, 512], F32, tag="mv")
                nc.vector.scalar_tensor_tensor(mv[:], ay[:], T22, ax[:], OP.mult, OP.is_ge)

                grads.append(grad)
                mhs.append(mh)
                mvs.append(mv)
                sds.append(sd)

            # pass B: NMS per tile (needs neighbor grad tiles)
            for t in range(NT):
                nv = min(VALID, H - VALID * t)
                grad = grads[t]
                gU = nmpool.tile([128, 514], F32, tag="gU")
                gD = nmpool.tile([128, 514], F32, tag="gD")
                if t == NT - 1:
                    nc.gpsimd.memset(gU[:], 0.0)
                else:
                    nc.gpsimd.memset(gU[:, 0:1], 0.0)
                    nc.gpsimd.memset(gU[:, 513:514], 0.0)
                if t == 0:
                    nc.gpsimd.memset(gD[:], 0.0)
                else:
                    nc.gpsimd.memset(gD[:, 0:1], 0.0)
                    nc.gpsimd.memset(gD[:, 513:514], 0.0)
                # gU[p] = grad(row+1); gD[p] = grad(row-1)
                nc.gpsimd.dma_start(gU[3:2 + nv, 1:513], grad[4:3 + nv, 1:513])
                if t < NT - 1:
                    nc.gpsimd.dma_start(gU[2 + nv:3 + nv, 1:513], grads[t + 1][3:4, 1:513])
                nc.gpsimd.dma_start(gD[4:3 + nv, 1:513], grad[3:2 + nv, 1:513])
                if t > 0:
                    nc.gpsimd.dma_start(gD[3:4, 1:513], grads[t - 1][124:125, 1:513])

                m0 = nmpool.tile([128, 512], F32, tag="m0")
                nc.vector.tensor_tensor(m0[:], grad[:, 0:512], grad[:, 2:514], OP.max)
                m2 = nmpool.tile([128, 512], F32, tag="m2")
                nc.vector.tensor_tensor(m2[:], gU[:, 1:513], gD[:, 1:513], OP.max)
                m1 = nmpool.tile([128, 512], F32, tag="m1")
                # gU[p] = grad(y+1) [down], gD[p] = grad(y-1) [up]
                # m1: (y+1,x+1),(y-1,x-1); m3: (y+1,x-1),(y-1,x+1)
                nc.vector.tensor_tensor(m1[:], gU[:, 2:514], gD[:, 0:512], OP.max)
                m3 = nmpool.tile([128, 512], F32, tag="m3")
                nc.vector.tensor_tensor(m3[:], gU[:, 0:512], gD[:, 2:514], OP.max)

                m = nmpool.tile([128, 512], F32, tag="m")
                nc.vector.tensor_copy(m[:], m3[:])
                nc.vector.copy_predicated(m[:], sds[t][:].bitcast(mybir.dt.int32), m1[:])
                nc.vector.copy_predicated(m[:], mvs[t][:].bitcast(mybir.dt.int32), m2[:])
                nc.vector.copy_predicated(m[:], mhs[t][:].bitcast(mybir.dt.int32), m0[:])
                c1 = nmpool.tile([128, 512], F32, tag="c1")
                nc.vector.tensor_tensor(c1[:], grad[:, 1:513], m[:], OP.is_gt)
                o01 = nmpool.tile([128, 512], F32, tag="o01")
                nc.vector.scalar_tensor_tensor(
                    o01[:], grad[:, 1:513], 2.0, c1[:], OP.is_ge, OP.logical_and)
                nc.gpsimd.dma_start(y[img, 0, VALID * t:VALID * t + nv, :], o01[3:3 + nv, :])
    _split_excess_waits(nc)
    return nc


_CACHE = {}


def kernel(img, gauss_w, sobel_w, dir_w):
    img = np.ascontiguousarray(np.asarray(img, dtype=np.float32))
    assert img.shape == (16, 3, H, W)
    key = "prog"
    if key not in _CACHE:
        _CACHE[key] = _build_program(gauss_w)
    nc = _CACHE[key]
    bands = _make_band_consts(gauss_w)
    in_maps = [
        {"x": img[2 * c:2 * c + 2], "bands": bands} for c in range(NCORES)
    ]
    res = run_bass_kernel_spmd(nc, in_maps, core_ids=list(range(NCORES)))
    out = np.concatenate([res.results[c]["y"] for c in range(NCORES)], axis=0)
    return out.astype(np.float32)


if __name__ == "__main__":
    img = np.load('/tmp/img.npy')
    dummy_g = np.exp(-0.5 * ((np.arange(5) - 2.0)) ** 2).astype(np.float32)
    out = kernel(img, dummy_g, None, None)
    ref = np.load('/tmp/ref_out.npy')
    n = (out != ref).sum()
    print("mismatch px:", n, " rel err:", np.linalg.norm(out - ref) / np.linalg.norm(ref))


# revision 16
# speedup vs baseline: 1.4553x; 1.4553x over previous
"""Canny edge detection (nn_Canny_56916906606715) on 8 Trainium2 NeuronCores.

Data-parallel over batch: 16 images -> 2 per core. Per core:
  - horizontal gaussian as shifted-AP FMAs on VectorE (3 channels batched)
  - sobel horizontal taps folded into TensorE matmuls on shifted Xh views:
    gx = A@Xh<x-1> - A@Xh<x+1>, gy = B@Xh<x-1> + 2B@Xh<x> + B@Xh<x+1>,
    with edge-exact staged vertical band matrices (fp32, PSUM accumulate);
    GX/GY from the channel-summed Xh the same way
  - Square/Sqrt/Abs/Sign/copies on ScalarE
  - NMS: sector masks (bf16) from (GX,GY) sign/tan tests; neighbor max via
    copy_predicated; vertical +-1 shifts via SBUF->SBUF DMA; both images
    batched along the free dimension; NMS of tile t-1 interleaved with tile t
Output is the binary 0/1 float mask, exactly mirroring the reference ordering
(threshold >= 2 AND strict local max along the quantized gradient direction).
"""
import numpy as np
from contextlib import ExitStack

import concourse.bass as bass
import concourse.tile as tile
from concourse import mybir
from concourse.bass_utils import run_bass_kernel_spmd

F32 = mybir.dt.float32
BF16 = mybir.dt.bfloat16
AF = mybir.ActivationFunctionType
OP = mybir.AluOpType

H = W = 512
VALID = 122          # valid output rows per conv tile (128 - 2*3)
NT = 5               # conv tiles per image (122*4 + 24 = 512)
NIMG = 2
NCH = 3
NCORES = 8
PI_REF = 3.14159     # reference uses this constant, not np.pi
NBAND = 4            # A, -A, B, 2B

_WS_CTR = [0]


def _split_excess_waits(nc):
    """This walrus build accepts a single sync wait per instruction. Split
    excess waits onto injected same-engine drains placed immediately before
    the overloaded instruction (waits then fire earlier - semantically safe)."""
    n_fixed = 0
    for f in nc.m.functions:
        for bb in f.blocks:
            insts = bb.instructions
            i = 0
            while i < len(insts):
                ins = insts[i]
                si = getattr(ins, 'sync_info', None)
                if si is not None and len(si.on_wait) > 1:
                    waits = list(si.on_wait)
                    ins.sync_info = mybir.SyncInfo(
                        on_wait=waits[:1], on_update=list(si.on_update))
                    rest = waits[1:]
                    new = []
                    for wchunk in rest:
                        d = mybir.InstDrain(
                            name=f"I-waitsplit-{_WS_CTR[0]}", ins=[], outs=[])
                        _WS_CTR[0] += 1
                        d.engine = ins.engine
                        d.sync_info = mybir.SyncInfo(on_wait=[wchunk], on_update=[])
                        new.append(d)
                    insts[i:i] = new
                    i += len(new)
                    n_fixed += 1
                i += 1
    return n_fixed


def _padded_band(taps, offs, N=H):
    M = np.zeros((N, N), dtype=np.float64)
    for tp, d in zip(taps, offs):
        idx = np.arange(max(0, -d), min(N, N - d))
        M[idx, idx + d] = tp
    return M


def _make_band_consts(gauss_w):
    """Per-tile lhsT band blocks [128, NBAND*NT*128]: (A, -A, B, 2B) x tile."""
    g = np.asarray(gauss_w, dtype=np.float32)
    g1 = np.float64(g[1])
    Gv = _padded_band(g.astype(np.float64), [-2, -1, 0, 1, 2])
    Sm = _padded_band([1, 2, 1], [-1, 0, 1])
    Dv = _padded_band([1, 0, -1], [-1, 0, 1])
    MA = ((Sm @ Gv) * g1).astype(np.float32)   # vertical operator for gx
    MB = ((Dv @ Gv) * g1).astype(np.float32)   # vertical operator for gy
    mats = (MA, -MA, MB, np.float32(2.0) * MB)
    out = np.zeros((128, NBAND * NT * 128), dtype=np.float32)
    for bi, Mfull in enumerate(mats):
        Mfull = Mfull.astype(np.float32)
        for t in range(NT):
            r0 = VALID * t - 3
            L = np.zeros((128, 128), dtype=np.float32)
            ks = np.arange(max(0, -r0), min(128, H - r0))
            for m in range(128):
                ro = r0 + m
                if 0 <= ro < H:
                    L[ks, m] = Mfull[ro, r0 + ks]
            out[:, (bi * NT + t) * 128:(bi * NT + t + 1) * 128] = L
    return out


def _iv(tile_obj, nblk, width, off, w):
    """AP over [128, nblk*width] tile: blocks i at [i*width+off : +w]."""
    return tile_obj[:].rearrange("p (i q) -> p i q", i=nblk)[:, :, off:off + w]


def _build_program(gauss_w):
    g = np.asarray(gauss_w, dtype=np.float32)
    c0 = float(np.float32(g[0] / g[1]))
    c2 = float(np.float32(1.0 / g[1]))
    T22 = float(np.float32(np.tan(PI_REF / 8)))

    nc = bass.Bass()
    x = nc.declare_dram_parameter("x", [NIMG, NCH, H, W], F32, isOutput=False)
    bands = nc.declare_dram_parameter(
        "bands", [128, NBAND * NT * 128], F32, isOutput=False)
    y = nc.declare_dram_parameter("y", [NIMG, 1, H, W], F32, isOutput=True)

    with tile.TileContext(nc) as tc, ExitStack() as ctx:
        cpool = ctx.enter_context(tc.tile_pool(name="consts", bufs=1))
        bt = cpool.tile([128, NBAND * NT * 128], F32)
        nc.gpsimd.dma_start(bt[:], bands[:])

        def band(bi, t):
            return bt[:, (bi * NT + t) * 128:(bi * NT + t + 1) * 128]

        xpool = ctx.enter_context(tc.tile_pool(name="xin", bufs=2))
        hbpool = ctx.enter_context(tc.tile_pool(name="hb", bufs=1))
        xhpool = ctx.enter_context(tc.tile_pool(name="xh", bufs=2))
        xspool = ctx.enter_context(tc.tile_pool(name="xhsum", bufs=2))
        sqpool = ctx.enter_context(tc.tile_pool(name="sq", bufs=2))
        mgpool = ctx.enter_context(tc.tile_pool(name="mg", bufs=2))
        gdpool = ctx.enter_context(tc.tile_pool(name="grad", bufs=4))
        axpool = ctx.enter_context(tc.tile_pool(name="axy", bufs=2))
        mkpool = ctx.enter_context(tc.tile_pool(name="mask", bufs=3))
        nmpool = ctx.enter_context(tc.tile_pool(name="nms", bufs=1))
        psA = ctx.enter_context(tc.tile_pool(name="psA", bufs=1, space="PSUM"))
        psB = ctx.enter_context(tc.tile_pool(name="psB", bufs=1, space="PSUM"))

        grads, mhs, mvs, sds = [], [], [], []

        def pass_a(t):
            r0 = VALID * t - 3
            nv = min(VALID, H - VALID * t)
            lo, hi = max(0, r0), min(H, r0 + 128)
            grad2 = gdpool.tile([128, 2 * 514], F32, tag="grad")
            for off in (0, 513, 514, 1027):
                nc.gpsimd.memset(grad2[:, off:off + 1], 0.0)

            mh2 = mkpool.tile([128, 1024], BF16, tag="mh")
            mv2 = mkpool.tile([128, 1024], BF16, tag="mv")
            sd2 = mkpool.tile([128, 1024], BF16, tag="sd")
            mag2b = mgpool.tile([128, 2 * 1536], F32, tag="mag")
            s2 = axpool.tile([128, 1024], F32, tag="s2")

            for img in range(NIMG):
                X3 = xpool.tile([128, 3 * 516], F32, tag="X")
                if t == 0 or t == NT - 1:
                    nc.gpsimd.memset(X3[:], 0.0)
                else:
                    nc.gpsimd.memset(_iv(X3, 3, 516, 0, 2), 0.0)
                    nc.gpsimd.memset(_iv(X3, 3, 516, 514, 2), 0.0)
                for c in range(NCH):
                    nc.gpsimd.dma_start(
                        X3[lo - r0:hi - r0, c * 516 + 2:c * 516 + 514],
                        x[img, c, lo:hi, :])

                # horizontal gaussian, 3 channels batched; Xh scaled by 1/g1
                t1 = hbpool.tile([128, 1536], F32, tag="t1")
                t2 = hbpool.tile([128, 1536], F32, tag="t2")
                nc.vector.tensor_tensor(
                    t1[:], _iv(X3, 3, 516, 0, 512), _iv(X3, 3, 516, 4, 512),
                    OP.add)
                nc.vector.tensor_tensor(
                    t2[:], _iv(X3, 3, 516, 1, 512), _iv(X3, 3, 516, 3, 512),
                    OP.add)
                nc.vector.scalar_tensor_tensor(
                    t1[:], t1[:], c0, t2[:], OP.mult, OP.add)
                Xh3 = xhpool.tile([128, 3 * 514], F32, tag="Xh")
                nc.gpsimd.memset(_iv(Xh3, 3, 514, 0, 1), 0.0)
                nc.gpsimd.memset(_iv(Xh3, 3, 514, 513, 1), 0.0)
                nc.vector.scalar_tensor_tensor(
                    _iv(Xh3, 3, 514, 1, 512), _iv(X3, 3, 516, 2, 512), c2,
                    t1[:], OP.mult, OP.add)

                # channel-summed Xh for GX/GY (guard cols stay zero)
                Xhs = xspool.tile([128, 514], F32, tag="xhsum")
                nc.vector.tensor_tensor(
                    Xhs[:], Xh3[:, 0:514], Xh3[:, 514:1028], OP.add)
                nc.vector.tensor_tensor(
                    Xhs[:], Xhs[:], Xh3[:, 1028:1542], OP.add)

                # vertical convs on TensorE with folded horizontal sobel taps
                gx3 = psA.tile([128, 1536], F32, tag="gx3")
                gy3 = psA.tile([128, 1536], F32, tag="gy3")
                for c in range(NCH):
                    s = c * 514
                    o = c * 512
                    nc.tensor.matmul(gx3[:, o:o + 512], band(0, t),
                                     Xh3[:, s + 0:s + 512], start=True, stop=False)
                    nc.tensor.matmul(gx3[:, o:o + 512], band(1, t),
                                     Xh3[:, s + 2:s + 514], start=False, stop=True)
                    nc.tensor.matmul(gy3[:, o:o + 512], band(2, t),
                                     Xh3[:, s + 0:s + 512], start=True, stop=False)
                    nc.tensor.matmul(gy3[:, o:o + 512], band(3, t),
                                     Xh3[:, s + 1:s + 513], start=False, stop=False)
                    nc.tensor.matmul(gy3[:, o:o + 512], band(2, t),
                                     Xh3[:, s + 2:s + 514], start=False, stop=True)
                GXY = psB.tile([128, 1024], F32, tag="GXY")
                nc.tensor.matmul(GXY[:, 0:512], band(0, t),
                                 Xhs[:, 0:512], start=True, stop=False)
                nc.tensor.matmul(GXY[:, 0:512], band(1, t),
                                 Xhs[:, 2:514], start=False, stop=True)
                nc.tensor.matmul(GXY[:, 512:1024], band(2, t),
                                 Xhs[:, 0:512], start=True, stop=False)
                nc.tensor.matmul(GXY[:, 512:1024], band(3, t),
                                 Xhs[:, 1:513], start=False, stop=False)
                nc.tensor.matmul(GXY[:, 512:1024], band(2, t),
                                 Xhs[:, 2:514], start=False, stop=True)

                sq = sqpool.tile([128, 3072], F32, tag="sq")
                nc.scalar.activation(sq[:, 0:1536], gx3[:], AF.Square)
                nc.scalar.activation(sq[:, 1536:3072], gy3[:], AF.Square)
                nc.vector.tensor_tensor(
                    mag2b[:, img * 1536:(img + 1) * 1536],
                    sq[:, 0:1536], sq[:, 1536:3072], OP.add)

                # sector mask ingredients for this image
                axy = axpool.tile([128, 1024], F32, tag="axy")
                nc.scalar.activation(axy[:, 0:512], GXY[:, 0:512], AF.Abs)
                nc.scalar.activation(axy[:, 512:1024], GXY[:, 512:1024], AF.Abs)
                sgx = axpool.tile([128, 512], F32, tag="sgx")
                nc.scalar.activation(sgx[:], GXY[:, 0:512], AF.Sign)
                nc.vector.tensor_tensor(
                    s2[:, img * 512:(img + 1) * 512], sgx[:],
                    GXY[:, 512:1024], OP.mult)
                nc.vector.scalar_tensor_tensor(
                    mh2[:, img * 512:(img + 1) * 512], axy[:, 0:512], T22,
                    axy[:, 512:1024], OP.mult, OP.is_ge)
                nc.vector.scalar_tensor_tensor(
                    mv2[:, img * 512:(img + 1) * 512], axy[:, 512:1024], T22,
                    axy[:, 0:512], OP.mult, OP.is_ge)
            nc.vector.tensor_single_scalar(sd2[:], s2[:], 0.0, OP.is_gt)

            # magnitudes and grad accumulation, both images batched
            nc.scalar.activation(mag2b[:], mag2b[:], AF.Sqrt)
            nc.vector.tensor_tensor(
                _iv(grad2, 2, 514, 1, 512),
                _iv(mag2b, 2, 1536, 0, 512),
                _iv(mag2b, 2, 1536, 512, 512), OP.add)
            nc.vector.tensor_tensor(
                _iv(grad2, 2, 514, 1, 512),
                _iv(grad2, 2, 514, 1, 512),
                _iv(mag2b, 2, 1536, 1024, 512), OP.add)

            grads.append(grad2)
            mhs.append(mh2)
            mvs.append(mv2)
            sds.append(sd2)

        def pass_b(t):
            nv = min(VALID, H - VALID * t)
            grad2 = grads[t]
            gU2 = nmpool.tile([128, 2 * 514], F32, tag="gU")
            gD2 = nmpool.tile([128, 2 * 514], F32, tag="gD")
            if t == NT - 1:
                nc.gpsimd.memset(gU2[:], 0.0)
            else:
                for off in (0, 513, 514, 1027):
                    nc.gpsimd.memset(gU2[:, off:off + 1], 0.0)
            if t == 0:
                nc.gpsimd.memset(gD2[:], 0.0)
            else:
                for off in (0, 513, 514, 1027):
                    nc.gpsimd.memset(gD2[:, off:off + 1], 0.0)
            for i in range(NIMG):
                o = i * 514
                nc.gpsimd.dma_start(gU2[3:2 + nv, o + 1:o + 513],
                                    grad2[4:3 + nv, o + 1:o + 513])
                if t < NT - 1:
                    nc.gpsimd.dma_start(gU2[2 + nv:3 + nv, o + 1:o + 513],
                                        grads[t + 1][3:4, o + 1:o + 513])
                nc.gpsimd.dma_start(gD2[4:3 + nv, o + 1:o + 513],
                                    grad2[3:2 + nv, o + 1:o + 513])
                if t > 0:
                    nc.gpsimd.dma_start(gD2[3:4, o + 1:o + 513],
                                        grads[t - 1][124:125, o + 1:o + 513])

            # gU2 holds grad(y+1) [down], gD2 holds grad(y-1) [up]
            # m1: (y+1,x+1),(y-1,x-1); m3: (y+1,x-1),(y-1,x+1)
            m1b = nmpool.tile([128, 1024], F32, tag="m1")
            nc.vector.tensor_tensor(
                m1b[:], _iv(gU2, 2, 514, 2, 512), _iv(gD2, 2, 514, 0, 512),
                OP.max)
            m3b = nmpool.tile([128, 1024], F32, tag="m3")
            nc.vector.tensor_tensor(
                m3b[:], _iv(gU2, 2, 514, 0, 512), _iv(gD2, 2, 514, 2, 512),
                OP.max)
            m0b = nmpool.tile([128, 1024], F32, tag="m0")
            nc.vector.tensor_tensor(
                m0b[:], _iv(grad2, 2, 514, 0, 512), _iv(grad2, 2, 514, 2, 512),
                OP.max)
            m2b = nmpool.tile([128, 1024], F32, tag="m2")
            nc.vector.tensor_tensor(
                m2b[:], _iv(gU2, 2, 514, 1, 512), _iv(gD2, 2, 514, 1, 512),
                OP.max)

            m = nmpool.tile([128, 1024], F32, tag="m")
            nc.scalar.copy(m[:], m3b[:])
            nc.vector.copy_predicated(
                m[:], sds[t][:].bitcast(mybir.dt.int16), m1b[:])
            nc.vector.copy_predicated(
                m[:], mvs[t][:].bitcast(mybir.dt.int16), m2b[:])
            nc.vector.copy_predicated(
                m[:], mhs[t][:].bitcast(mybir.dt.int16), m0b[:])
            c1 = nmpool.tile([128, 1024], F32, tag="c1")
            nc.vector.tensor_tensor(
                c1[:], _iv(grad2, 2, 514, 1, 512), m[:], OP.is_gt)
            o2 = nmpool.tile([128, 1024], F32, tag="o2")
            nc.vector.scalar_tensor_tensor(
                o2[:], _iv(grad2, 2, 514, 1, 512), 2.0, c1[:],
                OP.is_ge, OP.logical_and)
            for i in range(NIMG):
                nc.gpsimd.dma_start(
                    y[i, 0, VALID * t:VALID * t + nv, :],
                    o2[3:3 + nv, i * 512:(i + 1) * 512])

        for t in range(NT):
            pass_a(t)
            if t >= 1:
                pass_b(t - 1)
        pass_b(NT - 1)

    _split_excess_waits(nc)
    return nc


_CACHE = {}


def kernel(img, gauss_w, sobel_w, dir_w):
    img = np.ascontiguousarray(np.asarray(img, dtype=np.float32))
    assert img.shape == (16, 3, H, W)
    if "prog" not in _CACHE:
        _CACHE["prog"] = _build_program(gauss_w)
    nc = _CACHE["prog"]
    bands = _make_band_consts(gauss_w)
    in_maps = [
        {"x": img[2 * c:2 * c + 2], "bands": bands} for c in range(NCORES)
    ]
    res = run_bass_kernel_spmd(nc, in_maps, core_ids=list(range(NCORES)))
    out = np.concatenate([res.results[c]["y"] for c in range(NCORES)], axis=0)
    return out.astype(np.float32)
